# revision 36
# baseline (speedup 1.0000x reference)
"""Trainium2 Bass kernel for nn_ContextAttnDecoder (8 NeuronCores).

Strategy:
  - LSTM step + both attentions computed per-core (batch rows sharded 16/core
    for attention; LSTM replicated since it needs full h1 for the vocab matmul).
  - Vocab dimension of out_W sharded 8 ways (6272 padded cols/core); softmax
    denominator combined with a tiny AllReduce.
  - Pointer-copy scatter done per-core on its vocab slice via GPSIMD
    local_scatter (per-partition indices, host-precomputed, duplicate
    occurrences split into 3 rounds).
  - One small AllGather ships (1-p_gen)-scaled ctx attention scores + p_gen
    from the batch-owner cores to everyone.

All host-side work is input marshalling: slicing, transposing weight layouts,
and precomputing integer scatter maps from the (host-visible) index tensors.
"""

import sys
import numpy as np

if '/opt/trn_rl_repo' not in sys.path:
    sys.path.insert(0, '/opt/trn_rl_repo')

V, E, H, B, TENC, TCTX, OOV = 50000, 512, 512, 128, 128, 128, 50
NC_ = 8
W = 6272            # per-core padded vocab width; 8*6272 = 50176 >= 50050
CH = 1568           # local_scatter chunk width (4 chunks/core)
R = 3               # duplicate-occurrence rounds
BL = 16             # batch rows owned per core
MM_CHUNKS = [(i * 512, 512) for i in range(12)] + [(6144, 128)]  # 6272

_PROG = None        # cached (nc, meta) across calls


def _build_program():
    import concourse.bacc as bacc
    import concourse.bass as bass
    from concourse import mybir, library_config
    import concourse.tile as tile

    f32 = mybir.dt.float32
    f32r = mybir.dt.float32r
    f16 = mybir.dt.float16
    i16 = mybir.dt.int16
    Alu = mybir.AluOpType
    Act = mybir.ActivationFunctionType

    nc = bacc.Bacc(trn_type="TRN2", target_bir_lowering=False, debug=False,
                   num_devices=NC_)

    # ---------------- I/O ----------------
    def din(name, shape, dt=f32):
        return nc.dram_tensor(name, list(shape), dt, kind="ExternalInput")

    emb_t = din('emb_t', (512, 128))
    h0_t = din('h0_t', (512, 128))
    c0_b = din('c0_b', (128, 512))
    wih_aug = din('wih_aug', (513, 2048))
    whh = din('whh', (512, 2048))
    attn_wt = din('attn_wt', (513, 512))
    cattn_wt = din('cattn_wt', (513, 512))
    genw = din('genw', (512, 4))
    gbsig = din('gbsig', (1, BL))
    mk_in = din('mk_in', (1, BL))
    mf_in = din('mf_in', (1, BL))
    outwt = din('outwt', (513, W))
    enc_t = din('enc_t', (BL, 512, 128))
    ctx_t = din('ctx_t', (BL, 512, 128))
    skt = din('skt', (128, BL))
    embt_own = din('embt_own', (512, BL))
    idx_maps = din('idx_maps', (4 * R, 128, 128), i16)
    ident_in = din('ident_in', (128, 128))
    ones_row_in = din('ones_row_in', (1, 128))
    one_elem_in = din('one_elem_in', (1, 1))

    out_lp = nc.dram_tensor('out_lp', [128, W], f32, kind="ExternalOutput")
    h1_out = nc.dram_tensor('h1_out', [128, 512], f32, kind="ExternalOutput")
    c1_out = nc.dram_tensor('c1_out', [128, 512], f32, kind="ExternalOutput")

    with tile.TileContext(nc) as tc:
        with (
            tc.tile_pool(name="const", bufs=1) as constp,
            tc.tile_pool(name="acts", bufs=1) as acts,
            tc.tile_pool(name="wstream", bufs=6) as wstream,
            tc.tile_pool(name="gstream", bufs=4) as gstream,
            tc.tile_pool(name="estream", bufs=3) as estream,
            tc.tile_pool(name="small", bufs=2) as small,
            tc.tile_pool(name="big", bufs=1) as bigp,
            tc.tile_pool(name="ps_big", bufs=2, space="PSUM") as ps_big,
            tc.tile_pool(name="ps_sm", bufs=3, space="PSUM") as ps_sm,
            tc.tile_pool(name="ps_acc", bufs=1, space="PSUM") as ps_acc,
            tc.tile_pool(name="dram", bufs=1, space="DRAM") as dramp,
        ):
            # ---- constants ----
            ident = constp.tile([128, 128], f32)
            nc.sync.dma_start(ident[:], ident_in[:])
            ones_row = constp.tile([1, 128], f32)
            nc.sync.dma_start(ones_row[:].bitcast(f32r), ones_row_in[:].bitcast(f32r))
            one_elem = constp.tile([1, 1], f32)
            nc.sync.dma_start(one_elem[:].bitcast(f32r), one_elem_in[:].bitcast(f32r))
            ones_col = constp.tile([128, 1], f32)
            nc.sync.dma_start(ones_col[:].bitcast(f32r),
                              ones_row_in[:].rearrange("a b -> b a").bitcast(f32r))

            # gpsimd custom-op library (local_scatter) — load once up front
            nc.gpsimd.load_library(library_config.local_scatter)

            # ---- phase 1: LSTM step over full batch ----
            embt_sb = acts.tile([128, 4, 128], f32)
            nc.sync.dma_start(embt_sb[:].bitcast(f32r),
                              emb_t[:].rearrange("(k p) b -> p k b", p=128).bitcast(f32r))
            h0t_sb = acts.tile([128, 4, 128], f32)
            nc.sync.dma_start(h0t_sb[:].bitcast(f32r),
                              h0_t[:].rearrange("(k p) b -> p k b", p=128).bitcast(f32r))
            c0_sb = acts.tile([128, 512], f32)
            nc.sync.dma_start(c0_sb[:], c0_b[:])

            gate_func = [Act.Sigmoid, Act.Sigmoid, Act.Tanh, Act.Sigmoid]
            gate_sb = []  # sig_i, sig_f, tanh_g, sig_o
            for n in range(4):
                ps_g = ps_big.tile([128, 512], f32, tag="ps_mm")
                for kt in range(4):
                    rw = gstream.tile([128, 512], f32, tag="g_ih")
                    nc.sync.dma_start(
                        rw[:].bitcast(f32r),
                        wih_aug[kt * 128:(kt + 1) * 128,
                                n * 512:(n + 1) * 512].bitcast(f32r))
                    nc.tensor.matmul(ps_g[:], embt_sb[:, kt, :].bitcast(f32r),
                                     rw[:].bitcast(f32r), start=(kt == 0), stop=False)
                rb = gstream.tile([1, 512], f32, tag="g_bias")
                nc.sync.dma_start(rb[:].bitcast(f32r),
                                  wih_aug[512:513, n * 512:(n + 1) * 512].bitcast(f32r))
                nc.tensor.matmul(ps_g[:], ones_row[:].bitcast(f32r),
                                 rb[:].bitcast(f32r), start=False, stop=False)
                for kt in range(4):
                    rh = gstream.tile([128, 512], f32, tag="g_hh")
                    nc.sync.dma_start(
                        rh[:].bitcast(f32r),
                        whh[kt * 128:(kt + 1) * 128,
                            n * 512:(n + 1) * 512].bitcast(f32r))
                    nc.tensor.matmul(ps_g[:], h0t_sb[:, kt, :].bitcast(f32r),
                                     rh[:].bitcast(f32r), start=False, stop=(kt == 3))
                gact = acts.tile([128, 512], f32, tag=f"gate{n}", name=f"gate{n}")
                nc.scalar.activation(gact[:], ps_g[:], gate_func[n])
                gate_sb.append(gact)

            t_ig = acts.tile([128, 512], f32)
            nc.vector.tensor_tensor(t_ig[:], gate_sb[0][:], gate_sb[2][:], op=Alu.mult)
            t_fc = acts.tile([128, 512], f32)
            nc.vector.tensor_tensor(t_fc[:], gate_sb[1][:], c0_sb[:], op=Alu.mult)
            c1_sb = acts.tile([128, 512], f32)
            nc.vector.tensor_tensor(c1_sb[:], t_fc[:], t_ig[:], op=Alu.add)
            tanh_c1 = acts.tile([128, 512], f32)
            nc.scalar.activation(tanh_c1[:], c1_sb[:], Act.Tanh)
            h1_sb = acts.tile([128, 512], f32)
            nc.vector.tensor_tensor(h1_sb[:].bitcast(f32r), gate_sb[3][:],
                                    tanh_c1[:], op=Alu.mult)
            nc.sync.dma_start(c1_out[:], c1_sb[:])
            nc.sync.dma_start(h1_out[:], h1_sb[:])

            # h1^T tiles (K=h layout for matmul lhsT)
            h1t_sb = acts.tile([128, 4, 128], f32)
            for kt in range(4):
                ps_t = ps_sm.tile([128, 128], f32, tag="ps_sm")
                nc.tensor.transpose(ps_t[:], h1_sb[:, kt * 128:(kt + 1) * 128], ident[:])
                nc.vector.tensor_copy(h1t_sb[:, kt, :].bitcast(f32r), ps_t[:])

            # ---- phase 2: vocab logits + exp + per-core softmax denominator ----
            expl = bigp.tile([128, W], f32)
            sumexp_parts = acts.tile([128, len(MM_CHUNKS)], f32)
            for ci, (c0_, cn) in enumerate(MM_CHUNKS):
                ps_l = ps_big.tile([128, cn], f32, tag="ps_mm")
                for kt in range(4):
                    rw = wstream.tile([128, 512], f32, tag="w_out")
                    nc.sync.dma_start(
                        rw[:, :cn].bitcast(f32r),
                        outwt[kt * 128:(kt + 1) * 128, c0_:c0_ + cn].bitcast(f32r))
                    nc.tensor.matmul(ps_l[:], h1t_sb[:, kt, :].bitcast(f32r),
                                     rw[:, :cn].bitcast(f32r),
                                     start=(kt == 0), stop=False)
                rb = wstream.tile([1, 512], f32, tag="w_bias")
                nc.sync.dma_start(rb[:, :cn].bitcast(f32r),
                                  outwt[512:513, c0_:c0_ + cn].bitcast(f32r))
                nc.tensor.matmul(ps_l[:], ones_row[:].bitcast(f32r),
                                 rb[:, :cn].bitcast(f32r), start=False, stop=True)
                nc.scalar.activation(expl[:, c0_:c0_ + cn], ps_l[:], Act.Exp,
                                     accum_out=sumexp_parts[:, ci:ci + 1])
            denom_part = acts.tile([128, 1], f32)
            nc.vector.tensor_reduce(denom_part[:], sumexp_parts[:],
                                    axis=mybir.AxisListType.X, op=Alu.add)

            # ---- phase 3: attention on own 16 batch rows ----
            # h1T_own = h1^T restricted to this core's 16 batch columns
            skt_sb = acts.tile([128, BL], f32)
            nc.sync.dma_start(skt_sb[:].bitcast(f32r), skt[:].bitcast(f32r))
            ps_ho = ps_acc.tile([128, 4, BL], f32)
            for hc in range(4):
                nc.tensor.matmul(ps_ho[:, hc, :],
                                 h1_sb[:, hc * 128:(hc + 1) * 128].bitcast(f32r),
                                 skt_sb[:].bitcast(f32r), start=True, stop=True)
            ho_sb = acts.tile([128, 4, BL], f32)
            nc.vector.tensor_copy(ho_sb[:].bitcast(f32r), ps_ho[:])

            # dec^T = Wattn @ h1_aug^T_own   (h-major, own 16 batch cols)
            decT = {}
            for fl, wt_dram in (('e', attn_wt), ('c', cattn_wt)):
                awt = acts.tile([128, 4, 512], f32, name=f"awt_{fl}")
                nc.sync.dma_start(
                    awt[:].bitcast(f32r),
                    wt_dram[0:512, :].rearrange("(k p) n -> p k n", p=128).bitcast(f32r))
                awb = acts.tile([1, 512], f32, name=f"awb_{fl}")
                nc.sync.dma_start(awb[:].bitcast(f32r),
                                  wt_dram[512:513, :].bitcast(f32r))
                dT = acts.tile([128, 4, BL], f32, name=f"decT_{fl}")
                for mc in range(4):
                    ps_d = ps_sm.tile([128, BL], f32, tag="ps_sm")
                    for kt in range(4):
                        nc.tensor.matmul(
                            ps_d[:],
                            awt[:, kt, mc * 128:(mc + 1) * 128].bitcast(f32r),
                            ho_sb[:, kt, :].bitcast(f32r),
                            start=(kt == 0), stop=False)
                    nc.tensor.matmul(ps_d[:],
                                     awb[:, mc * 128:(mc + 1) * 128].bitcast(f32r),
                                     ones_row[:, 0:BL].bitcast(f32r),
                                     start=False, stop=True)
                    nc.vector.tensor_copy(dT[:, mc, :].bitcast(f32r), ps_d[:])
                decT[fl] = dT

            genw_sb = acts.tile([128, 4, 4], f32)
            nc.sync.dma_start(
                genw_sb[:].bitcast(f32r),
                genw[:].rearrange("(k p) c -> p k c", p=128).bitcast(f32r))

            expS = {'e': acts.tile([128, BL], f32, name="expS_e"),
                    'c': acts.tile([128, BL], f32, name="expS_c")}
            # A/B accumulators: one PSUM tile, cols b; [0]=enc-term, [1]=ctx-term
            ps_AB = ps_acc.tile([1, 2, BL], f32)
            for b in range(BL):
                for fi, (fl, src) in enumerate((('e', enc_t), ('c', ctx_t))):
                    et = estream.tile([128, 4, 128], f32, tag=f"et_{fl}")
                    nc.sync.dma_start(
                        et[:].bitcast(f32r),
                        src[b].rearrange("(k p) t -> p k t", p=128).bitcast(f32r))
                    ps_s = ps_sm.tile([128, 1], f32, tag="ps_sm")
                    for kt in range(4):
                        nc.tensor.matmul(ps_s[:], et[:, kt, :],
                                         decT[fl][:, kt, b:b + 1],
                                         start=(kt == 0), stop=(kt == 3))
                    nc.scalar.activation(expS[fl][:, b:b + 1].bitcast(f32r),
                                         ps_s[:], Act.Exp)
                    # w1e = enc_t[b] rows dotted with gen_W quarter (T-vector)
                    ps_w = ps_sm.tile([128, 1], f32, tag="ps_sm")
                    for kt in range(4):
                        nc.tensor.matmul(ps_w[:], et[:, kt, :],
                                         genw_sb[:, kt, fi:fi + 1],
                                         start=(kt == 0), stop=(kt == 3))
                    w1e = small.tile([128, 1], f32, tag="w1e")
                    nc.vector.tensor_copy(w1e[:].bitcast(f32r), ps_w[:])
                    # A[b] = sum_t expS[t,b] * w1e[t]
                    nc.tensor.matmul(ps_AB[:, fi, b:b + 1], w1e[:],
                                     expS[fl][:, b:b + 1],
                                     start=True, stop=True)

            # Z rows (sum over t of expS)
            Z_sb = {}
            for fl in ('e', 'c'):
                ps_z = ps_sm.tile([1, BL], f32, tag="ps_sm")
                nc.tensor.matmul(ps_z[:], ones_col[:],
                                 expS[fl][:], start=True, stop=True)
                zt = small.tile([1, BL], f32, tag=f"z_{fl}", name=f"z_{fl}")
                nc.vector.tensor_copy(zt[:], ps_z[:])
                Z_sb[fl] = zt

            # embT_own  (h-major, 16 own batch cols)
            eo_sb = acts.tile([128, 4, BL], f32)
            nc.sync.dma_start(
                eo_sb[:].bitcast(f32r),
                embt_own[:].rearrange("(k p) c -> p k c", p=128).bitcast(f32r))

            # C = gW3.h1_own + gW4.emb_own  (1 x BL)
            ps_C = ps_sm.tile([1, BL], f32, tag="ps_sm")
            for hc in range(4):
                nc.tensor.matmul(ps_C[:], genw_sb[:, hc, 2:3],
                                 ho_sb[:, hc, :],
                                 start=(hc == 0), stop=False)
            for hc in range(4):
                nc.tensor.matmul(ps_C[:], genw_sb[:, hc, 3:4],
                                 eo_sb[:, hc, :],
                                 start=False, stop=(hc == 3))

            gbsig_sb = small.tile([1, BL], f32, tag="row1")
            nc.sync.dma_start(gbsig_sb[:], gbsig[:])
            mk_sb = small.tile([1, BL], f32, tag="row1")
            nc.sync.dma_start(mk_sb[:], mk_in[:])
            mf_sb = small.tile([1, BL], f32, tag="row1")
            nc.sync.dma_start(mf_sb[:], mf_in[:])

            z1r = small.tile([1, BL], f32, tag="row2")
            nc.vector.reciprocal(z1r[:], Z_sb['e'][:])
            z2r = small.tile([1, BL], f32, tag="row2")
            with nc.allow_low_precision(reason="f32r round-off for matmul operand"):
                nc.vector.reciprocal(z2r[:].bitcast(f32r), Z_sb['c'][:])
            glog = small.tile([1, BL], f32, tag="row3")
            nc.vector.scalar_tensor_tensor(glog[:], ps_AB[:, 0, :], 1.0, z1r[:],
                                           op0=Alu.mult, op1=Alu.mult)
            gB = small.tile([1, BL], f32, tag="row3")
            nc.vector.tensor_tensor(gB[:], ps_AB[:, 1, :], z2r[:], op=Alu.mult)
            nc.vector.tensor_tensor(glog[:], glog[:], gB[:], op=Alu.add)
            cC = small.tile([1, BL], f32, tag="row3")
            nc.vector.tensor_tensor(cC[:], ps_C[:], gbsig_sb[:], op=Alu.add)
            nc.vector.tensor_tensor(glog[:], glog[:], cC[:], op=Alu.add)
            pg_row = small.tile([1, BL], f32, tag="row4")
            nc.scalar.activation(pg_row[:].bitcast(f32r), glog[:], Act.Sigmoid)
            nc.vector.tensor_tensor(pg_row[:].bitcast(f32r), pg_row[:], mk_sb[:],
                                    op=Alu.mult)
            nc.vector.tensor_tensor(pg_row[:].bitcast(f32r), pg_row[:], mf_sb[:],
                                    op=Alu.add)
            ompg_row = small.tile([1, BL], f32, tag="row4")
            nc.scalar.activation(ompg_row[:].bitcast(f32r), pg_row[:], Act.Copy,
                                 bias=1.0, scale=-1.0)

            # transpose (1,BL) rows -> (BL,1) cols via K=1 matmuls
            cols = {}
            for nm, row in (('pg', pg_row), ('ompg', ompg_row), ('z2r', z2r)):
                ps_cl = ps_sm.tile([BL, 1], f32, tag="ps_sm")
                nc.tensor.matmul(ps_cl[:], row[:],
                                 one_elem[:], start=True, stop=True)
                ct_ = small.tile([BL, 1], f32, tag=f"col_{nm}", name=f"col_{nm}")
                nc.vector.tensor_copy(ct_[:], ps_cl[:])
                cols[nm] = ct_

            # own scores, batch-major, scaled: (1-pg)/Z2 * expS_c
            ps_e2b = ps_sm.tile([BL, 128], f32, tag="ps_sm")
            nc.tensor.transpose(ps_e2b[:], expS['c'][:], ident[:])
            s2p_sb = small.tile([BL, 128], f32, tag="s2p")
            nc.vector.tensor_scalar(s2p_sb[:], ps_e2b[:], cols['z2r'][:],
                                    cols['ompg'][:], op0=Alu.mult, op1=Alu.mult)

            # ---- collective 1: AllGather payload (BL,130) -> (128,130) ----
            ag_in = dramp.tile([BL, 129], f32)
            ag_out = dramp.tile([128, 129], f32, addr_space="Shared")
            nc.sync.dma_start(ag_in[:, 0:1], cols['pg'][:])
            nc.sync.dma_start(ag_in[:, 1:129], s2p_sb[:])
            nc.gpsimd.collective_compute(
                "AllGather", mybir.AluOpType.bypass,
                replica_groups=[list(range(NC_))],
                ins=[ag_in[:].opt()], outs=[ag_out[:].opt()])

            pg_all = acts.tile([128, 1], f32)
            nc.sync.dma_start(pg_all[:], ag_out[:, 0:1])
            s2all = acts.tile([128, 128], f32)
            nc.sync.dma_start(s2all[:], ag_out[:, 1:129])
            s2f16 = acts.tile([128, 128], f16)
            nc.vector.tensor_copy(s2f16[:], s2all[:])

            # ---- scatter rounds into corr ----
            corr0 = bigp.tile([128, W], f16)
            for ch in range(4):
                ix0 = small.tile([128, 128], i16, tag="ix")
                nc.sync.dma_start(ix0[:], idx_maps[ch * R + 0])
                nc.gpsimd.local_scatter(
                    out_ap=corr0[:, ch * CH:(ch + 1) * CH], data_ap=s2f16[:],
                    idxs_ap=ix0[:], channels=128, num_elems=CH, num_idxs=128)
            for ch in range(4):
                for r in (1, 2):
                    ixr = small.tile([128, 128], i16, tag="ix")
                    nc.sync.dma_start(ixr[:], idx_maps[ch * R + r])
                    tmp = small.tile([128, CH], f16, tag="corr_tmp")
                    nc.gpsimd.local_scatter(
                        out_ap=tmp[:], data_ap=s2f16[:], idxs_ap=ixr[:],
                        channels=128, num_elems=CH, num_idxs=128)
                    nc.vector.tensor_tensor(corr0[:, ch * CH:(ch + 1) * CH],
                                            corr0[:, ch * CH:(ch + 1) * CH],
                                            tmp[:], op=Alu.add)

            # ---- collective 2: AllReduce softmax denominator ----
            ar_in = dramp.tile([128, 1], f32)
            ar_out = dramp.tile([128, 1], f32, addr_space="Shared")
            nc.sync.dma_start(ar_in[:], denom_part[:])
            nc.gpsimd.collective_compute(
                "AllReduce", mybir.AluOpType.add,
                replica_groups=[list(range(NC_))],
                ins=[ar_in[:].opt()], outs=[ar_out[:].opt()])
            denom_sum = acts.tile([128, 1], f32)
            nc.sync.dma_start(denom_sum[:], ar_out[:])

            # ---- tail: P = expl * (pg/denom) + corr ; log(clip) ; out ----
            dr = acts.tile([128, 1], f32)
            nc.vector.reciprocal(dr[:], denom_sum[:])
            c_sb = acts.tile([128, 1], f32)
            nc.vector.tensor_tensor(c_sb[:], pg_all[:], dr[:], op=Alu.mult)
            for sl in range(4):
                s = slice(sl * CH, (sl + 1) * CH)
                nc.vector.scalar_tensor_tensor(expl[:, s], expl[:, s], c_sb[:],
                                               corr0[:, s], op0=Alu.mult, op1=Alu.add)
                nc.vector.tensor_scalar_max(expl[:, s], expl[:, s], 1e-10)
                nc.scalar.activation(expl[:, s], expl[:, s], Act.Ln)
                nc.sync.dma_start(out_lp[:, s], expl[:, s])

    nc.compile()
    return nc


def _host_inputs(inputs):
    """Build per-core input maps from the full-size problem inputs."""
    inp = {k: np.asarray(v) for k, v in inputs.items()}
    input_ids = inp['input_ids']
    h0 = np.asarray(inp['h0'], np.float32)
    c0 = np.asarray(inp['c0'], np.float32)
    enc = np.asarray(inp['encoder_outputs'], np.float32)
    ctxo = np.asarray(inp['context_type_outputs'], np.float32)
    ctxv = np.asarray(inp['context_type_variable'])
    emb_tab = np.asarray(inp['embedding'], np.float32)
    W_ih = np.asarray(inp['W_ih'], np.float32)
    W_hh = np.asarray(inp['W_hh'], np.float32)
    b_ih = np.asarray(inp['b_ih'], np.float32)
    b_hh = np.asarray(inp['b_hh'], np.float32)
    attn_W = np.asarray(inp['attn_W'], np.float32)
    attn_b = np.asarray(inp['attn_b'], np.float32)
    ctx_attn_W = np.asarray(inp['ctx_attn_W'], np.float32)
    ctx_attn_b = np.asarray(inp['ctx_attn_b'], np.float32)
    gen_W = np.asarray(inp['gen_W'], np.float32)
    gen_b = np.asarray(inp['gen_b'], np.float32)
    sig_bias = np.asarray(inp['sig_bias'], np.float32)
    out_W = np.asarray(inp['out_W'], np.float32)
    out_b = np.asarray(inp['out_b'], np.float32)

    emb = emb_tab[input_ids[:, 0]]                       # B,E
    ctx_len = (ctxv > 0).sum(1)
    mask_keep = (ctx_len > 0).astype(np.float32)
    mask_force = 1.0 - mask_keep

    c = np.ascontiguousarray
    emb_t = c(emb.T)
    h0_t = c(h0[0].T)
    wih_aug = c(np.vstack([W_ih.T, (b_ih + b_hh)[None, :]]))
    whh_t = c(W_hh.T)
    attn_wt = c(np.vstack([attn_W.T, attn_b[None, :]]))
    cattn_wt = c(np.vstack([ctx_attn_W.T, ctx_attn_b[None, :]]))
    genw = c(gen_W[0].reshape(4, 512).T)                 # (512,4) cols gW1..gW4

    # vocab-sharded out_W^T with bias row; -80 bias kills padded/OOV columns
    outwt_full = np.full((513, NC_ * W), 0.0, np.float32)
    outwt_full[512, :] = -80.0
    outwt_full[:512, :V] = out_W.T
    outwt_full[512, :V] = out_b

    # scatter maps
    idx_maps = np.full((NC_, 4 * R, 128, 128), -1, np.int16)
    for b in range(B):
        occ = {}
        row = ctxv[b]
        for t in range(TCTX):
            v = int(row[t])
            r = occ.get(v, 0)
            occ[v] = r + 1
            if r >= R:
                raise RuntimeError(f"duplicate-occurrence overflow at b={b} v={v}")
            k, off = divmod(v, W)
            ch, loc = divmod(off, CH)
            idx_maps[k, ch * R + r, b, t] = loc

    ident = np.eye(128, dtype=np.float32)
    ones_row = np.ones((1, 128), np.float32)
    one_elem = np.ones((1, 1), np.float32)
    gbsig = np.full((1, BL), float(gen_b[0] + sig_bias[0]), np.float32)

    in_maps = []
    for k in range(NC_):
        rows = slice(BL * k, BL * (k + 1))
        skt = np.zeros((128, BL), np.float32)
        skt[np.arange(BL * k, BL * (k + 1)), np.arange(BL)] = 1.0
        m = {
            'emb_t': emb_t, 'h0_t': h0_t, 'c0_b': c(c0[0]),
            'wih_aug': wih_aug, 'whh': whh_t,
            'attn_wt': attn_wt, 'cattn_wt': cattn_wt,
            'genw': genw, 'gbsig': gbsig,
            'mk_in': c(mask_keep[None, rows]), 'mf_in': c(mask_force[None, rows]),
            'outwt': c(outwt_full[:, k * W:(k + 1) * W]),
            'enc_t': c(enc[rows].transpose(0, 2, 1)),
            'ctx_t': c(ctxo[rows].transpose(0, 2, 1)),
            'skt': skt, 'embt_own': c(emb[rows].T),
            'idx_maps': c(idx_maps[k]),
            'ident_in': ident, 'ones_row_in': ones_row, 'one_elem_in': one_elem,
        }
        in_maps.append(m)
    return in_maps


def kernel(**inputs):
    global _PROG
    from concourse.bass_utils import run_bass_kernel_spmd

    if _PROG is None:
        _PROG = _build_program()
    nc = _PROG

    in_maps = _host_inputs(inputs)
    res = run_bass_kernel_spmd(nc, in_maps, core_ids=list(range(NC_))).results

    out_full = np.concatenate([res[k]['out_lp'] for k in range(NC_)], axis=1)
    lp = np.ascontiguousarray(out_full[:, :V + OOV])
    h1 = res[0]['h1_out'][None]
    c1 = res[0]['c1_out'][None]
    return lp, h1, c1


if __name__ == '__main__':
    d = np.load('/tmp/inputs.npz')
    lp, h1, c1 = kernel(**{k: d[k] for k in d.files})
    e = np.load('/tmp/expected.npz')
    for nm, a, b in (('lp', lp, e['lp']), ('h1', h1, e['h1']), ('c1', c1, e['c1'])):
        err = np.abs(a - b).max()
        print(nm, 'absmax', err, 'scale', np.abs(b).max())


# revision 39
# speedup vs baseline: 1.0143x; 1.0143x over previous
"""Trainium2 Bass kernel for nn_ContextAttnDecoder (8 NeuronCores).

Strategy:
  - LSTM step + both attentions computed per-core (batch rows sharded 16/core
    for attention; LSTM replicated since it needs full h1 for the vocab matmul).
  - Vocab dimension of out_W sharded 8 ways (6272 padded cols/core); softmax
    denominator combined with a tiny AllReduce.
  - Pointer-copy scatter done per-core on its vocab slice via GPSIMD
    local_scatter (per-partition indices, host-precomputed, duplicate
    occurrences split into 3 rounds).
  - One small AllGather ships (1-p_gen)-scaled ctx attention scores + p_gen
    from the batch-owner cores to everyone.

All host-side work is input marshalling: slicing, transposing weight layouts,
and precomputing integer scatter maps from the (host-visible) index tensors.
"""

import sys
import numpy as np

if '/opt/trn_rl_repo' not in sys.path:
    sys.path.insert(0, '/opt/trn_rl_repo')

V, E, H, B, TENC, TCTX, OOV = 50000, 512, 512, 128, 128, 128, 50
NC_ = 8
W = 6272            # per-core padded vocab width; 8*6272 = 50176 >= 50050
CH = 1568           # local_scatter chunk width (4 chunks/core)
R = 3               # duplicate-occurrence rounds
BL = 16             # batch rows owned per core
MM_CHUNKS = [(i * 512, 512) for i in range(12)] + [(6144, 128)]  # 6272

_PROG = None        # cached (nc, meta) across calls


def _build_program():
    import concourse.bacc as bacc
    import concourse.bass as bass
    from concourse import mybir, library_config
    import concourse.tile as tile

    f32 = mybir.dt.float32
    f32r = mybir.dt.float32r
    f16 = mybir.dt.float16
    i16 = mybir.dt.int16
    Alu = mybir.AluOpType
    Act = mybir.ActivationFunctionType

    nc = bacc.Bacc(trn_type="TRN2", target_bir_lowering=False, debug=False,
                   num_devices=NC_)

    # ---------------- I/O ----------------
    def din(name, shape, dt=f32):
        return nc.dram_tensor(name, list(shape), dt, kind="ExternalInput")

    emb_t = din('emb_t', (512, 128))
    h0_t = din('h0_t', (512, 128))
    c0_b = din('c0_b', (128, 512))
    wih_aug = din('wih_aug', (513, 2048))
    whh = din('whh', (512, 2048))
    attn_wt = din('attn_wt', (513, 512))
    cattn_wt = din('cattn_wt', (513, 512))
    genw = din('genw', (512, 4))
    gbsig = din('gbsig', (1, BL))
    mk_in = din('mk_in', (1, BL))
    mf_in = din('mf_in', (1, BL))
    outwt = din('outwt', (513, W), f16)
    enc_t = din('enc_t', (BL, 512, 128))
    ctx_t = din('ctx_t', (BL, 512, 128))
    skt = din('skt', (128, BL))
    embt_own = din('embt_own', (512, BL))
    idx_maps = din('idx_maps', (4 * R, 128, 128), i16)
    ident_in = din('ident_in', (128, 128))
    ones_row_in = din('ones_row_in', (1, 128))
    one_elem_in = din('one_elem_in', (1, 1))

    out_lp = nc.dram_tensor('out_lp', [128, W], f32, kind="ExternalOutput")
    h1_out = nc.dram_tensor('h1_out', [128, 512], f32, kind="ExternalOutput")
    c1_out = nc.dram_tensor('c1_out', [128, 512], f32, kind="ExternalOutput")

    with tile.TileContext(nc) as tc:
        with (
            tc.tile_pool(name="const", bufs=1) as constp,
            tc.tile_pool(name="acts", bufs=1) as acts,
            tc.tile_pool(name="wstream", bufs=6) as wstream,
            tc.tile_pool(name="gstream", bufs=4) as gstream,
            tc.tile_pool(name="estream", bufs=3) as estream,
            tc.tile_pool(name="small", bufs=2) as small,
            tc.tile_pool(name="big", bufs=1) as bigp,
            tc.tile_pool(name="ps_big", bufs=2, space="PSUM") as ps_big,
            tc.tile_pool(name="ps_sm", bufs=3, space="PSUM") as ps_sm,
            tc.tile_pool(name="ps_acc", bufs=1, space="PSUM") as ps_acc,
            tc.tile_pool(name="dram", bufs=1, space="DRAM") as dramp,
        ):
            # ---- constants ----
            ident = constp.tile([128, 128], f32)
            nc.sync.dma_start(ident[:], ident_in[:])
            ones_row = constp.tile([1, 128], f32)
            nc.sync.dma_start(ones_row[:].bitcast(f32r), ones_row_in[:].bitcast(f32r))
            one_elem = constp.tile([1, 1], f32)
            nc.sync.dma_start(one_elem[:].bitcast(f32r), one_elem_in[:].bitcast(f32r))
            ones_col = constp.tile([128, 1], f32)
            nc.sync.dma_start(ones_col[:].bitcast(f32r),
                              ones_row_in[:].rearrange("a b -> b a").bitcast(f32r))

            # gpsimd custom-op library (local_scatter) — load once up front
            nc.gpsimd.load_library(library_config.local_scatter)

            # ---- phase 1: LSTM step over full batch ----
            embt_sb = acts.tile([128, 4, 128], f32)
            nc.sync.dma_start(embt_sb[:].bitcast(f32r),
                              emb_t[:].rearrange("(k p) b -> p k b", p=128).bitcast(f32r))
            h0t_sb = acts.tile([128, 4, 128], f32)
            nc.sync.dma_start(h0t_sb[:].bitcast(f32r),
                              h0_t[:].rearrange("(k p) b -> p k b", p=128).bitcast(f32r))
            c0_sb = acts.tile([128, 512], f32)
            nc.sync.dma_start(c0_sb[:], c0_b[:])

            gate_func = [Act.Sigmoid, Act.Sigmoid, Act.Tanh, Act.Sigmoid]
            gate_sb = []  # sig_i, sig_f, tanh_g, sig_o
            for n in range(4):
                ps_g = ps_big.tile([128, 512], f32, tag="ps_mm")
                for kt in range(4):
                    rw = gstream.tile([128, 512], f32, tag="g_ih")
                    nc.sync.dma_start(
                        rw[:].bitcast(f32r),
                        wih_aug[kt * 128:(kt + 1) * 128,
                                n * 512:(n + 1) * 512].bitcast(f32r))
                    nc.tensor.matmul(ps_g[:], embt_sb[:, kt, :].bitcast(f32r),
                                     rw[:].bitcast(f32r), start=(kt == 0), stop=False)
                rb = gstream.tile([1, 512], f32, tag="g_bias")
                nc.sync.dma_start(rb[:].bitcast(f32r),
                                  wih_aug[512:513, n * 512:(n + 1) * 512].bitcast(f32r))
                nc.tensor.matmul(ps_g[:], ones_row[:].bitcast(f32r),
                                 rb[:].bitcast(f32r), start=False, stop=False)
                for kt in range(4):
                    rh = gstream.tile([128, 512], f32, tag="g_hh")
                    nc.sync.dma_start(
                        rh[:].bitcast(f32r),
                        whh[kt * 128:(kt + 1) * 128,
                            n * 512:(n + 1) * 512].bitcast(f32r))
                    nc.tensor.matmul(ps_g[:], h0t_sb[:, kt, :].bitcast(f32r),
                                     rh[:].bitcast(f32r), start=False, stop=(kt == 3))
                gact = acts.tile([128, 512], f32, tag=f"gate{n}", name=f"gate{n}")
                nc.scalar.activation(gact[:], ps_g[:], gate_func[n])
                gate_sb.append(gact)

            t_ig = acts.tile([128, 512], f32)
            nc.vector.tensor_tensor(t_ig[:], gate_sb[0][:], gate_sb[2][:], op=Alu.mult)
            t_fc = acts.tile([128, 512], f32)
            nc.vector.tensor_tensor(t_fc[:], gate_sb[1][:], c0_sb[:], op=Alu.mult)
            c1_sb = acts.tile([128, 512], f32)
            nc.vector.tensor_tensor(c1_sb[:], t_fc[:], t_ig[:], op=Alu.add)
            tanh_c1 = acts.tile([128, 512], f32)
            nc.scalar.activation(tanh_c1[:], c1_sb[:], Act.Tanh)
            h1_sb = acts.tile([128, 512], f32)
            nc.vector.tensor_tensor(h1_sb[:].bitcast(f32r), gate_sb[3][:],
                                    tanh_c1[:], op=Alu.mult)
            nc.sync.dma_start(c1_out[:], c1_sb[:])
            nc.sync.dma_start(h1_out[:], h1_sb[:])

            # h1^T tiles (K=h layout for matmul lhsT)
            h1t_sb = acts.tile([128, 4, 128], f32)
            for kt in range(4):
                ps_t = ps_sm.tile([128, 128], f32, tag="ps_sm")
                nc.tensor.transpose(ps_t[:], h1_sb[:, kt * 128:(kt + 1) * 128], ident[:])
                nc.vector.tensor_copy(h1t_sb[:, kt, :].bitcast(f32r), ps_t[:])

            # ---- phase 2: vocab logits + exp + per-core softmax denominator ----
            # fp16 weights/activations (fp32 PSUM accumulate): halves outwt DMA
            # and runs the PE at full rate vs the 4-pass fp32 modes.
            h1t16 = acts.tile([128, 4, 128], f16)
            nc.vector.tensor_copy(h1t16[:], h1t_sb[:])
            ones16 = constp.tile([1, 128], f16)
            nc.vector.tensor_copy(ones16[:], ones_row[:])
            expl = bigp.tile([128, W], f32)
            sumexp_parts = acts.tile([128, len(MM_CHUNKS)], f32)
            for ci, (c0_, cn) in enumerate(MM_CHUNKS):
                ps_l = ps_big.tile([128, cn], f32, tag="ps_mm")
                for kt in range(4):
                    rw = wstream.tile([128, 512], f16, tag="w_out")
                    nc.sync.dma_start(
                        rw[:, :cn], outwt[kt * 128:(kt + 1) * 128, c0_:c0_ + cn])
                    nc.tensor.matmul(ps_l[:], h1t16[:, kt, :], rw[:, :cn],
                                     start=(kt == 0), stop=False)
                rb = wstream.tile([1, 512], f16, tag="w_bias")
                nc.sync.dma_start(rb[:, :cn], outwt[512:513, c0_:c0_ + cn])
                nc.tensor.matmul(ps_l[:], ones16[:], rb[:, :cn],
                                 start=False, stop=True)
                nc.scalar.activation(expl[:, c0_:c0_ + cn], ps_l[:], Act.Exp,
                                     accum_out=sumexp_parts[:, ci:ci + 1])
            denom_part = acts.tile([128, 1], f32)
            nc.vector.tensor_reduce(denom_part[:], sumexp_parts[:],
                                    axis=mybir.AxisListType.X, op=Alu.add)

            # ---- phase 3: attention on own 16 batch rows ----
            # h1T_own = h1^T restricted to this core's 16 batch columns
            skt_sb = acts.tile([128, BL], f32)
            nc.sync.dma_start(skt_sb[:].bitcast(f32r), skt[:].bitcast(f32r))
            ps_ho = ps_acc.tile([128, 4, BL], f32)
            for hc in range(4):
                nc.tensor.matmul(ps_ho[:, hc, :],
                                 h1_sb[:, hc * 128:(hc + 1) * 128].bitcast(f32r),
                                 skt_sb[:].bitcast(f32r), start=True, stop=True)
            ho_sb = acts.tile([128, 4, BL], f32)
            nc.vector.tensor_copy(ho_sb[:].bitcast(f32r), ps_ho[:])

            # dec^T = Wattn @ h1_aug^T_own   (h-major, own 16 batch cols)
            decT = {}
            for fl, wt_dram in (('e', attn_wt), ('c', cattn_wt)):
                awt = acts.tile([128, 4, 512], f32, name=f"awt_{fl}")
                nc.sync.dma_start(
                    awt[:].bitcast(f32r),
                    wt_dram[0:512, :].rearrange("(k p) n -> p k n", p=128).bitcast(f32r))
                awb = acts.tile([1, 512], f32, name=f"awb_{fl}")
                nc.sync.dma_start(awb[:].bitcast(f32r),
                                  wt_dram[512:513, :].bitcast(f32r))
                dT = acts.tile([128, 4, BL], f32, name=f"decT_{fl}")
                for mc in range(4):
                    ps_d = ps_sm.tile([128, BL], f32, tag="ps_sm")
                    for kt in range(4):
                        nc.tensor.matmul(
                            ps_d[:],
                            awt[:, kt, mc * 128:(mc + 1) * 128].bitcast(f32r),
                            ho_sb[:, kt, :].bitcast(f32r),
                            start=(kt == 0), stop=False)
                    nc.tensor.matmul(ps_d[:],
                                     awb[:, mc * 128:(mc + 1) * 128].bitcast(f32r),
                                     ones_row[:, 0:BL].bitcast(f32r),
                                     start=False, stop=True)
                    nc.vector.tensor_copy(dT[:, mc, :].bitcast(f32r), ps_d[:])
                decT[fl] = dT

            genw_sb = acts.tile([128, 4, 4], f32)
            nc.sync.dma_start(
                genw_sb[:].bitcast(f32r),
                genw[:].rearrange("(k p) c -> p k c", p=128).bitcast(f32r))

            expS = {'e': acts.tile([128, BL], f32, name="expS_e"),
                    'c': acts.tile([128, BL], f32, name="expS_c")}
            # A/B accumulators: one PSUM tile, cols b; [0]=enc-term, [1]=ctx-term
            ps_AB = ps_acc.tile([1, 2, BL], f32)
            for b in range(BL):
                for fi, (fl, src) in enumerate((('e', enc_t), ('c', ctx_t))):
                    et = estream.tile([128, 4, 128], f32, tag=f"et_{fl}")
                    nc.sync.dma_start(
                        et[:].bitcast(f32r),
                        src[b].rearrange("(k p) t -> p k t", p=128).bitcast(f32r))
                    ps_s = ps_sm.tile([128, 1], f32, tag="ps_sm")
                    for kt in range(4):
                        nc.tensor.matmul(ps_s[:], et[:, kt, :],
                                         decT[fl][:, kt, b:b + 1],
                                         start=(kt == 0), stop=(kt == 3))
                    nc.scalar.activation(expS[fl][:, b:b + 1].bitcast(f32r),
                                         ps_s[:], Act.Exp)
                    # w1e = enc_t[b] rows dotted with gen_W quarter (T-vector)
                    ps_w = ps_sm.tile([128, 1], f32, tag="ps_sm")
                    for kt in range(4):
                        nc.tensor.matmul(ps_w[:], et[:, kt, :],
                                         genw_sb[:, kt, fi:fi + 1],
                                         start=(kt == 0), stop=(kt == 3))
                    w1e = small.tile([128, 1], f32, tag="w1e")
                    nc.vector.tensor_copy(w1e[:].bitcast(f32r), ps_w[:])
                    # A[b] = sum_t expS[t,b] * w1e[t]
                    nc.tensor.matmul(ps_AB[:, fi, b:b + 1], w1e[:],
                                     expS[fl][:, b:b + 1],
                                     start=True, stop=True)

            # Z rows (sum over t of expS)
            Z_sb = {}
            for fl in ('e', 'c'):
                ps_z = ps_sm.tile([1, BL], f32, tag="ps_sm")
                nc.tensor.matmul(ps_z[:], ones_col[:],
                                 expS[fl][:], start=True, stop=True)
                zt = small.tile([1, BL], f32, tag=f"z_{fl}", name=f"z_{fl}")
                nc.vector.tensor_copy(zt[:], ps_z[:])
                Z_sb[fl] = zt

            # embT_own  (h-major, 16 own batch cols)
            eo_sb = acts.tile([128, 4, BL], f32)
            nc.sync.dma_start(
                eo_sb[:].bitcast(f32r),
                embt_own[:].rearrange("(k p) c -> p k c", p=128).bitcast(f32r))

            # C = gW3.h1_own + gW4.emb_own  (1 x BL)
            ps_C = ps_sm.tile([1, BL], f32, tag="ps_sm")
            for hc in range(4):
                nc.tensor.matmul(ps_C[:], genw_sb[:, hc, 2:3],
                                 ho_sb[:, hc, :],
                                 start=(hc == 0), stop=False)
            for hc in range(4):
                nc.tensor.matmul(ps_C[:], genw_sb[:, hc, 3:4],
                                 eo_sb[:, hc, :],
                                 start=False, stop=(hc == 3))

            gbsig_sb = small.tile([1, BL], f32, tag="row1")
            nc.sync.dma_start(gbsig_sb[:], gbsig[:])
            mk_sb = small.tile([1, BL], f32, tag="row1")
            nc.sync.dma_start(mk_sb[:], mk_in[:])
            mf_sb = small.tile([1, BL], f32, tag="row1")
            nc.sync.dma_start(mf_sb[:], mf_in[:])

            z1r = small.tile([1, BL], f32, tag="row2")
            nc.vector.reciprocal(z1r[:], Z_sb['e'][:])
            z2r = small.tile([1, BL], f32, tag="row2")
            with nc.allow_low_precision(reason="f32r round-off for matmul operand"):
                nc.vector.reciprocal(z2r[:].bitcast(f32r), Z_sb['c'][:])
            glog = small.tile([1, BL], f32, tag="row3")
            nc.vector.scalar_tensor_tensor(glog[:], ps_AB[:, 0, :], 1.0, z1r[:],
                                           op0=Alu.mult, op1=Alu.mult)
            gB = small.tile([1, BL], f32, tag="row3")
            nc.vector.tensor_tensor(gB[:], ps_AB[:, 1, :], z2r[:], op=Alu.mult)
            nc.vector.tensor_tensor(glog[:], glog[:], gB[:], op=Alu.add)
            cC = small.tile([1, BL], f32, tag="row3")
            nc.vector.tensor_tensor(cC[:], ps_C[:], gbsig_sb[:], op=Alu.add)
            nc.vector.tensor_tensor(glog[:], glog[:], cC[:], op=Alu.add)
            pg_row = small.tile([1, BL], f32, tag="row4")
            nc.scalar.activation(pg_row[:].bitcast(f32r), glog[:], Act.Sigmoid)
            nc.vector.tensor_tensor(pg_row[:].bitcast(f32r), pg_row[:], mk_sb[:],
                                    op=Alu.mult)
            nc.vector.tensor_tensor(pg_row[:].bitcast(f32r), pg_row[:], mf_sb[:],
                                    op=Alu.add)
            ompg_row = small.tile([1, BL], f32, tag="row4")
            nc.scalar.activation(ompg_row[:].bitcast(f32r), pg_row[:], Act.Copy,
                                 bias=1.0, scale=-1.0)

            # transpose (1,BL) rows -> (BL,1) cols via K=1 matmuls
            cols = {}
            for nm, row in (('pg', pg_row), ('ompg', ompg_row), ('z2r', z2r)):
                ps_cl = ps_sm.tile([BL, 1], f32, tag="ps_sm")
                nc.tensor.matmul(ps_cl[:], row[:],
                                 one_elem[:], start=True, stop=True)
                ct_ = small.tile([BL, 1], f32, tag=f"col_{nm}", name=f"col_{nm}")
                nc.vector.tensor_copy(ct_[:], ps_cl[:])
                cols[nm] = ct_

            # own scores, batch-major, scaled: (1-pg)/Z2 * expS_c
            ps_e2b = ps_sm.tile([BL, 128], f32, tag="ps_sm")
            nc.tensor.transpose(ps_e2b[:], expS['c'][:], ident[:])
            s2p_sb = small.tile([BL, 128], f32, tag="s2p")
            nc.vector.tensor_scalar(s2p_sb[:], ps_e2b[:], cols['z2r'][:],
                                    cols['ompg'][:], op0=Alu.mult, op1=Alu.mult)

            # ---- collective 1: AllGather payload (BL,130) -> (128,130) ----
            ag_in = dramp.tile([BL, 129], f32)
            ag_out = dramp.tile([128, 129], f32, addr_space="Shared")
            nc.sync.dma_start(ag_in[:, 0:1], cols['pg'][:])
            nc.sync.dma_start(ag_in[:, 1:129], s2p_sb[:])
            nc.gpsimd.collective_compute(
                "AllGather", mybir.AluOpType.bypass,
                replica_groups=[list(range(NC_))],
                ins=[ag_in[:].opt()], outs=[ag_out[:].opt()])

            pg_all = acts.tile([128, 1], f32)
            nc.sync.dma_start(pg_all[:], ag_out[:, 0:1])
            s2all = acts.tile([128, 128], f32)
            nc.sync.dma_start(s2all[:], ag_out[:, 1:129])
            s2f16 = acts.tile([128, 128], f16)
            nc.vector.tensor_copy(s2f16[:], s2all[:])

            # ---- scatter rounds into corr ----
            corr0 = bigp.tile([128, W], f16)
            for ch in range(4):
                ix0 = small.tile([128, 128], i16, tag="ix")
                nc.sync.dma_start(ix0[:], idx_maps[ch * R + 0])
                nc.gpsimd.local_scatter(
                    out_ap=corr0[:, ch * CH:(ch + 1) * CH], data_ap=s2f16[:],
                    idxs_ap=ix0[:], channels=128, num_elems=CH, num_idxs=128)
            for ch in range(4):
                for r in (1, 2):
                    ixr = small.tile([128, 128], i16, tag="ix")
                    nc.sync.dma_start(ixr[:], idx_maps[ch * R + r])
                    tmp = small.tile([128, CH], f16, tag="corr_tmp")
                    nc.gpsimd.local_scatter(
                        out_ap=tmp[:], data_ap=s2f16[:], idxs_ap=ixr[:],
                        channels=128, num_elems=CH, num_idxs=128)
                    nc.vector.tensor_tensor(corr0[:, ch * CH:(ch + 1) * CH],
                                            corr0[:, ch * CH:(ch + 1) * CH],
                                            tmp[:], op=Alu.add)

            # ---- collective 2: AllReduce softmax denominator ----
            ar_in = dramp.tile([128, 1], f32)
            ar_out = dramp.tile([128, 1], f32, addr_space="Shared")
            nc.sync.dma_start(ar_in[:], denom_part[:])
            nc.gpsimd.collective_compute(
                "AllReduce", mybir.AluOpType.add,
                replica_groups=[list(range(NC_))],
                ins=[ar_in[:].opt()], outs=[ar_out[:].opt()])
            denom_sum = acts.tile([128, 1], f32)
            nc.sync.dma_start(denom_sum[:], ar_out[:])

            # ---- tail: P = expl * (pg/denom) + corr ; log(clip) ; out ----
            dr = acts.tile([128, 1], f32)
            nc.vector.reciprocal(dr[:], denom_sum[:])
            c_sb = acts.tile([128, 1], f32)
            nc.vector.tensor_tensor(c_sb[:], pg_all[:], dr[:], op=Alu.mult)
            for sl in range(4):
                s = slice(sl * CH, (sl + 1) * CH)
                nc.vector.scalar_tensor_tensor(expl[:, s], expl[:, s], c_sb[:],
                                               corr0[:, s], op0=Alu.mult, op1=Alu.add)
                nc.vector.tensor_scalar_max(expl[:, s], expl[:, s], 1e-10)
                nc.scalar.activation(expl[:, s], expl[:, s], Act.Ln)
                nc.sync.dma_start(out_lp[:, s], expl[:, s])

    nc.compile()
    return nc


def _host_inputs(inputs):
    """Build per-core input maps from the full-size problem inputs."""
    inp = {k: np.asarray(v) for k, v in inputs.items()}
    input_ids = inp['input_ids']
    h0 = np.asarray(inp['h0'], np.float32)
    c0 = np.asarray(inp['c0'], np.float32)
    enc = np.asarray(inp['encoder_outputs'], np.float32)
    ctxo = np.asarray(inp['context_type_outputs'], np.float32)
    ctxv = np.asarray(inp['context_type_variable'])
    emb_tab = np.asarray(inp['embedding'], np.float32)
    W_ih = np.asarray(inp['W_ih'], np.float32)
    W_hh = np.asarray(inp['W_hh'], np.float32)
    b_ih = np.asarray(inp['b_ih'], np.float32)
    b_hh = np.asarray(inp['b_hh'], np.float32)
    attn_W = np.asarray(inp['attn_W'], np.float32)
    attn_b = np.asarray(inp['attn_b'], np.float32)
    ctx_attn_W = np.asarray(inp['ctx_attn_W'], np.float32)
    ctx_attn_b = np.asarray(inp['ctx_attn_b'], np.float32)
    gen_W = np.asarray(inp['gen_W'], np.float32)
    gen_b = np.asarray(inp['gen_b'], np.float32)
    sig_bias = np.asarray(inp['sig_bias'], np.float32)
    out_W = np.asarray(inp['out_W'], np.float32)
    out_b = np.asarray(inp['out_b'], np.float32)

    emb = emb_tab[input_ids[:, 0]]                       # B,E
    ctx_len = (ctxv > 0).sum(1)
    mask_keep = (ctx_len > 0).astype(np.float32)
    mask_force = 1.0 - mask_keep

    c = np.ascontiguousarray
    emb_t = c(emb.T)
    h0_t = c(h0[0].T)
    wih_aug = c(np.vstack([W_ih.T, (b_ih + b_hh)[None, :]]))
    whh_t = c(W_hh.T)
    attn_wt = c(np.vstack([attn_W.T, attn_b[None, :]]))
    cattn_wt = c(np.vstack([ctx_attn_W.T, ctx_attn_b[None, :]]))
    genw = c(gen_W[0].reshape(4, 512).T)                 # (512,4) cols gW1..gW4

    # vocab-sharded out_W^T with bias row; -80 bias kills padded/OOV columns
    outwt_full = np.full((513, NC_ * W), 0.0, np.float32)
    outwt_full[512, :] = -80.0
    outwt_full[:512, :V] = out_W.T
    outwt_full[512, :V] = out_b

    # scatter maps
    idx_maps = np.full((NC_, 4 * R, 128, 128), -1, np.int16)
    for b in range(B):
        occ = {}
        row = ctxv[b]
        for t in range(TCTX):
            v = int(row[t])
            r = occ.get(v, 0)
            occ[v] = r + 1
            if r >= R:
                raise RuntimeError(f"duplicate-occurrence overflow at b={b} v={v}")
            k, off = divmod(v, W)
            ch, loc = divmod(off, CH)
            idx_maps[k, ch * R + r, b, t] = loc

    ident = np.eye(128, dtype=np.float32)
    ones_row = np.ones((1, 128), np.float32)
    one_elem = np.ones((1, 1), np.float32)
    gbsig = np.full((1, BL), float(gen_b[0] + sig_bias[0]), np.float32)

    in_maps = []
    for k in range(NC_):
        rows = slice(BL * k, BL * (k + 1))
        skt = np.zeros((128, BL), np.float32)
        skt[np.arange(BL * k, BL * (k + 1)), np.arange(BL)] = 1.0
        m = {
            'emb_t': emb_t, 'h0_t': h0_t, 'c0_b': c(c0[0]),
            'wih_aug': wih_aug, 'whh': whh_t,
            'attn_wt': attn_wt, 'cattn_wt': cattn_wt,
            'genw': genw, 'gbsig': gbsig,
            'mk_in': c(mask_keep[None, rows]), 'mf_in': c(mask_force[None, rows]),
            'outwt': c(outwt_full[:, k * W:(k + 1) * W]).astype(np.float16),
            'enc_t': c(enc[rows].transpose(0, 2, 1)),
            'ctx_t': c(ctxo[rows].transpose(0, 2, 1)),
            'skt': skt, 'embt_own': c(emb[rows].T),
            'idx_maps': c(idx_maps[k]),
            'ident_in': ident, 'ones_row_in': ones_row, 'one_elem_in': one_elem,
        }
        in_maps.append(m)
    return in_maps


def kernel(**inputs):
    global _PROG
    from concourse.bass_utils import run_bass_kernel_spmd

    if _PROG is None:
        _PROG = _build_program()
    nc = _PROG

    in_maps = _host_inputs(inputs)
    res = run_bass_kernel_spmd(nc, in_maps, core_ids=list(range(NC_))).results

    out_full = np.concatenate([res[k]['out_lp'] for k in range(NC_)], axis=1)
    lp = np.ascontiguousarray(out_full[:, :V + OOV])
    h1 = res[0]['h1_out'][None]
    c1 = res[0]['c1_out'][None]
    return lp, h1, c1


if __name__ == '__main__':
    d = np.load('/tmp/inputs.npz')
    lp, h1, c1 = kernel(**{k: d[k] for k in d.files})
    e = np.load('/tmp/expected.npz')
    for nm, a, b in (('lp', lp, e['lp']), ('h1', h1, e['h1']), ('c1', c1, e['c1'])):
        err = np.abs(a - b).max()
        print(nm, 'absmax', err, 'scale', np.abs(b).max())
